# revision 53
# baseline (speedup 1.0000x reference)
"""Causal multi-head attention (B=2, S=2048, D=768, H=12) on 8 TRN2 NeuronCores.

Sharding: core c handles batch c//4, heads 3*(c%4) .. 3*(c%4)+3.

Per core (bf16 operands, fp8e4m3 DoubleRow PV/V-proj, fp32 PSUM):
  - q/k projections bf16, chunk-major across 6 parallel PSUM groups at
    startup so PE rides the serial input-DMA stream.
  - v projection fp8 DoubleRow (256-deep contraction, 0.5 cyc/row); per-head
    128-col groups laid out [ones | 63 pad | v(64)] so the denominator lands
    on ctx partition 0 (reciprocal_approx_fast silently misreads PSUM at
    nonzero partition offsets) and v sits 64-aligned (>32-partition reads
    must be 0/64-aligned). Blocks 0-1 run bf16 (block 0 feeds the q<128
    patch; block 1 hides the x8 DMA arrival).
  - scores TRANSPOSED sT[k,q] = K.Q^T bf16; causal mask ACCUMULATED into the
    diagonal 128-block via identity-lhsT matmul; odd chunks of a k-pair get
    a rank-1 -1e9 fill so the pair-exp writes exact zeros. Exactly one
    start=True per PSUM bank (start zeroes the whole 2KB region) and
    stop=True on each byte-range's final write.
  - exp on ACT straight from PSUM -> P^T fp8 in [128,2,QS] pair tiles; PV is
    fp8 DoubleRow over k-pairs, emitted one pair late so it never waits on
    exp. q<128 is patched with a bf16 exp + bf16 PV (fp8 softmax-weight
    error spikes at tiny Neff). Remaining projections/outproj work is
    slot-scheduled into the attention loop as PE fillers.
  - normalization: DVE reciprocal_approx_fast reads the denominator row from
    PSUM; bf16 copy on GpSimd; ones-matmul broadcast; DVE multiply -> bf16
    ctn. The PE-visible stage is deferred two pairs to hide the chain.
  - out-projection bf16; g0's units interleave into g1 as fillers; outputs
    stored bf16 on alternating HWDGE queues, summed across cores on host.
"""

import numpy as np
import ml_dtypes

B, S, D, H, HD = 2, 2048, 768, 12, 64
NH = 3                      # heads per core
NCORES = 8
SCALE = 1.0 / np.sqrt(HD)
QS = 1024                   # q superblock width
NG = S // QS                # 2 q superblocks
NKC = S // 128              # 16 k chunks
NXC = D // 128              # 6 contraction chunks of 128 over D
NEG = -1.0e9                # causal mask fill (exp(NEG*SCALE) == 0)

_cache = {}
DEBUG = False


def _enable_ldw_opt():
    """No-op: walrus's ldw-opt rejects the explicit InstLdweights that
    bf16/fp8 matmuls emit (it only composes with f32r's implicit loads)."""


def _build(reps=1):
    _enable_ldw_opt()
    key = ("nc", reps)
    if key in _cache:
        return _cache[key]
    import concourse.bacc as bacc
    import concourse.mybir as mybir
    import concourse.tile as tile

    f32 = mybir.dt.float32
    bf16 = mybir.dt.bfloat16
    fp8 = mybir.dt.float8e4
    DR = mybir.MatmulPerfMode.DoubleRow
    Exp = mybir.ActivationFunctionType.Exp
    add_op = mybir.AluOpType.add

    nc = bacc.Bacc(None, target_bir_lowering=False, debug=False, num_devices=NCORES)

    xT_d = nc.dram_tensor("xT", [D, S], bf16, kind="ExternalInput")
    x8T_d = nc.dram_tensor("x8T", [D, S], fp8, kind="ExternalInput")
    wqT_d = nc.dram_tensor("wqT", [D, NH * HD], bf16, kind="ExternalInput")
    wkT_d = nc.dram_tensor("wkT", [D, NH * HD], bf16, kind="ExternalInput")
    wv8T_d = nc.dram_tensor("wv8T", [D, 384], fp8, kind="ExternalInput")
    wvT_d = nc.dram_tensor("wvT", [D, 384], bf16, kind="ExternalInput")
    woT_d = nc.dram_tensor("woT", [128, 2, D], bf16, kind="ExternalInput")
    bq01_d = nc.dram_tensor("bq01", [128, 1], f32, kind="ExternalInput")
    bq2_d = nc.dram_tensor("bq2", [64, 1], f32, kind="ExternalInput")
    bk01_d = nc.dram_tensor("bk01", [128, 1], f32, kind="ExternalInput")
    bk2_d = nc.dram_tensor("bk2", [64, 1], f32, kind="ExternalInput")
    bv_d = nc.dram_tensor("bv", [1, 384], bf16, kind="ExternalInput")
    maskneg_d = nc.dram_tensor("maskneg", [128, 128], bf16, kind="ExternalInput")
    ident_d = nc.dram_tensor("ident", [128, 128], bf16, kind="ExternalInput")
    outT_d = nc.dram_tensor("outT", [D, S], bf16, kind="ExternalOutput")
    dbg = {}
    if DEBUG:
        import concourse.mybir as _mb
        dbg["dpt0"] = nc.dram_tensor("dpt0", [128, 2, QS], fp8, kind="ExternalOutput")
        dbg["dv8_0"] = nc.dram_tensor("dv8_0", [128, 2, NH, 128], fp8, kind="ExternalOutput")
        dbg["dv0b"] = nc.dram_tensor("dv0b", [128, NH, 128], bf16, kind="ExternalOutput")
        dbg["drec0"] = nc.dram_tensor("drec0", [1, 512], f32, kind="ExternalOutput")
        dbg["drec1"] = nc.dram_tensor("drec1", [1, 512], f32, kind="ExternalOutput")
        dbg["dq0"] = nc.dram_tensor("dq0", [128, 2, 512], bf16, kind="ExternalOutput")
        dbg["dk0"] = nc.dram_tensor("dk0", [128, 2, 512], bf16, kind="ExternalOutput")
        dbg["dctn0"] = nc.dram_tensor("dctn0", [128, 2, 512], bf16, kind="ExternalOutput")
        dbg["dcts0"] = nc.dram_tensor("dcts0", [64, 512], bf16, kind="ExternalOutput")
        dbg["dden0"] = nc.dram_tensor("dden0", [1, 512], f32, kind="ExternalOutput")
        dbg["dpt0b"] = nc.dram_tensor("dpt0b", [128, 128], bf16, kind="ExternalOutput")
        dbg["dctn1"] = nc.dram_tensor("dctn1", [128, 2, 512], bf16, kind="ExternalOutput")

    with tile.TileContext(nc) as tc:
        with (
            tc.tile_pool(name="const", bufs=1) as cst,
            tc.tile_pool(name="work", bufs=3) as wrk,
            tc.tile_pool(name="norm", bufs=2) as nrm,
            tc.tile_pool(name="ps_sT", bufs=2, space="PSUM") as ps_sT,
            tc.tile_pool(name="ps_ctx", bufs=1, space="PSUM") as ps_ctx,
            tc.tile_pool(name="ps_mm", bufs=2, space="PSUM") as ps_mm,
        ):
         for _rep in range(reps):
              # ---- constant / persistent SBUF ----
              # Small consts on the gpsimd SWDGE queue (its ~1.1us/DMA serial
              # generation cost is fine for these); weights + x interleaved
              # across the two HWDGE queues (SP, ACT) ordered by first use.
              maskneg_sb = cst.tile([128, 128], bf16)
              nc.gpsimd.dma_start(maskneg_sb[:], maskneg_d[:])
              ident_sb = cst.tile([128, 128], bf16)
              nc.gpsimd.dma_start(ident_sb[:], ident_d[:])
              bq01 = cst.tile([128, 1], f32)
              nc.gpsimd.dma_start(bq01[:], bq01_d[:])
              bq2 = cst.tile([64, 1], f32)
              nc.gpsimd.dma_start(bq2[:], bq2_d[:])
              bk01 = cst.tile([128, 1], f32)
              nc.gpsimd.dma_start(bk01[:], bk01_d[:])
              bk2 = cst.tile([64, 1], f32)
              nc.gpsimd.dma_start(bk2[:], bk2_d[:])
              bv_sb = cst.tile([1, 384], bf16)
              nc.gpsimd.dma_start(bv_sb[:], bv_d[:])

              # DMA engines serialize at HBM bandwidth, so submission order ==
              # arrival order: weights first, then x chunk-by-chunk (the
              # chunk-major initial projections ride this stream), then the
              # late-use tensors (wv bf16, wo, x8, wv8).
              xT_r = xT_d[:].rearrange("(c p) s -> p c s", p=128)
              x8_r = x8T_d[:].rearrange("(c p) s -> p c s", p=128)
              wq_sb = cst.tile([128, NXC, NH * HD], bf16)
              nc.sync.dma_start(wq_sb[:], wqT_d[:].rearrange("(c p) m -> p c m", p=128))
              wk_sb = cst.tile([128, NXC, NH * HD], bf16)
              nc.scalar.dma_start(wk_sb[:], wkT_d[:].rearrange("(c p) m -> p c m", p=128))
              x_sb = []
              for c in range(NXC):
                  xc = cst.tile([128, S], bf16, tag=f"x{c}")
                  eng = nc.sync if c % 2 == 0 else nc.scalar
                  eng.dma_start(xc[:], xT_r[:, c, :])
                  x_sb.append(xc)
              wv_sb = cst.tile([128, NXC, 384], bf16)
              nc.scalar.dma_start(wv_sb[:], wvT_d[:].rearrange("(c p) m -> p c m", p=128))
              x8_sb = cst.tile([128, NXC, S], fp8)
              nc.sync.dma_start(x8_sb[:, 0:3, :], x8_r[:, 0:3, :])
              nc.scalar.dma_start(x8_sb[:, 3:6, :], x8_r[:, 3:6, :])
              wv8_sb = cst.tile([128, NXC, 384], fp8)
              nc.sync.dma_start(wv8_sb[:], wv8T_d[:].rearrange("(c p) m -> p c m", p=128))
              wo_sb = cst.tile([128, 2, D], bf16)
              nc.scalar.dma_start(wo_sb[:], woT_d[:])

              ones_f = cst.tile([1, 128], f32)
              nc.vector.memset(ones_f[:], 1.0)
              ones_b = cst.tile([1, 128], bf16)
              nc.vector.tensor_copy(ones_b[:], ones_f[:])
              ones_r = cst.tile([1, 128], mybir.dt.float32r)
              nc.vector.tensor_copy(ones_r[:], ones_f[:])
              negrow_f = cst.tile([1, 128], f32)
              nc.vector.memset(negrow_f[:], NEG)
              negrow_b = cst.tile([1, 128], bf16)
              nc.vector.tensor_copy(negrow_b[:], negrow_f[:])

              # persistent activations:
              # q/k: one tile per 512-wide s-super; slot 0 holds heads 0/1
              # stacked on partitions, slot 1 head 2.
              qk_sb = {
                  t: [
                      cst.tile(
                          [128, 2, 512], bf16, tag=f"{t}sp{sp}", name=f"{t}sp{sp}"
                      )
                      for sp in range(4)
                  ]
                  for t in ("q", "k")
              }
              # v_aug fp8 per k-pair: [128, 2, NH, 65]; col 64 of each head
              # group is the ones column (from the bias matmul's bv layout).
              v8_sb = [
                  cst.tile([128, 2, NH, 128], fp8, tag=f"vp{i}", name=f"vp{i}")
                  for i in range(NKC // 2)
              ]
              # bf16 copy of v block 0 for the q<128 patch
              v0b_sb = cst.tile([128, NH, 128], bf16)

              def head_ap(t, h, lo, hi):
                  """AP for head h, global columns [lo, hi) (within one super)."""
                  sp, o = lo // 512, lo % 512
                  tile_ = qk_sb[t][sp]
                  if h < 2:
                      return tile_[64 * h : 64 * h + 64, 0, o : o + hi - lo]
                  return tile_[0:64, 1, o : o + hi - lo]

              def qk_proj(t, sp, mi):
                  scols = slice(512 * sp, 512 * sp + 512)
                  m0, msz, slot = ((0, 128, 0), (128, 64, 1))[mi]
                  b01, b2 = (bq01, bq2) if t == "q" else (bk01, bk2)
                  p = ps_mm.tile([128, 512], f32, tag="mm")
                  for c in range(NXC):
                      nc.tensor.matmul(
                          p[:msz, :],
                          (wq_sb if t == "q" else wk_sb)[:, c, m0 : m0 + msz],
                          x_sb[c][:, scols],
                          start=(c == 0),
                          stop=(c == NXC - 1),
                      )
                  nc.vector.tensor_scalar(
                      out=qk_sb[t][sp][:msz, slot, :],
                      in0=p[:msz, :],
                      scalar1=(b01 if mi == 0 else b2)[:msz],
                      scalar2=None,
                      op0=add_op,
                  )

              def v_proj(blk):
                  # fp8 DoubleRow over 3 contraction pairs; the bf16 rank-1
                  # bias matmul adds bv + the ones column of each head group.
                  # Block 0 runs bf16: it feeds the q<128 patch, whose whole
                  # point is bf16-quality v for the tiny-Neff positions.
                  p = ps_mm.tile([128, 384], f32, tag="mm")
                  if blk < 2:
                      for c in range(NXC):
                          nc.tensor.matmul(
                              p[:],
                              x_sb[c][:, 128 * blk : 128 * blk + 128],
                              wv_sb[:, c, :],
                              start=(c == 0),
                              stop=False,
                          )
                  else:
                      for i in range(3):
                          nc.tensor.matmul(
                              p[:],
                              x8_sb[:, 2 * i : 2 * i + 2, 128 * blk : 128 * blk + 128],
                              wv8_sb[:, 2 * i : 2 * i + 2, :],
                              start=(i == 0),
                              stop=False,
                              perf_mode=DR,
                          )
                  nc.tensor.matmul(p[:], ones_b[:], bv_sb[:], start=False, stop=True)
                  nc.vector.tensor_copy(
                      v8_sb[blk // 2][:, blk % 2, :, :], p[:, 0 : NH * 128]
                  )
                  if blk == 0:
                      nc.vector.tensor_copy(v0b_sb[:], p[:, 0 : NH * 128])

              # ---- initial projections: just enough for g=0 h=0 ----
              # Chunk-major across 6 parallel PSUM accumulation groups
              # (borrowing slots from every pool) so PE consumes each x chunk
              # the moment it lands instead of stalling group-by-group.
              igroups = [
                  ("q", 0, 0), ("q", 1, 0), ("k", 0, 0),
                  ("q", 1, 1), ("q", 0, 1), ("k", 0, 1),
              ]
              ipsum = [
                  ps_mm.tile([128, 512], f32, tag="mm", name="ip0"),
                  ps_mm.tile([128, 512], f32, tag="mm", name="ip1"),
                  ps_sT.tile([128, QS], f32, tag="sT", name="ip2"),
                  ps_sT.tile([128, QS], f32, tag="sT", name="ip3"),
                  ps_ctx.tile([128, 512], f32, tag="ctx0", name="ip4"),
                  ps_ctx.tile([128, 512], f32, tag="ctx1", name="ip5"),
              ]
              for c in range(NXC):
                  for gi, (t, sp, mi) in enumerate(igroups):
                      m0, msz, _ = ((0, 128, 0), (128, 64, 1))[mi]
                      nc.tensor.matmul(
                          ipsum[gi][:msz, 0:512],
                          (wq_sb if t == "q" else wk_sb)[:, c, m0 : m0 + msz],
                          x_sb[c][:, 512 * sp : 512 * sp + 512],
                          start=(c == 0),
                          stop=(c == NXC - 1),
                      )
              for gi, (t, sp, mi) in enumerate(igroups):
                  m0, msz, slot = ((0, 128, 0), (128, 64, 1))[mi]
                  b01, b2 = (bq01, bq2) if t == "q" else (bk01, bk2)
                  nc.vector.tensor_scalar(
                      out=qk_sb[t][sp][:msz, slot, :],
                      in0=ipsum[gi][:msz, 0:512],
                      scalar1=(b01 if mi == 0 else b2)[:msz],
                      scalar2=None,
                      op0=add_op,
                  )
              qk_proj("k", 1, 0)

              # remaining projection + g0-outproj work, interleaved into the
              # attention loop at fixed chunk slots (slot = global chunk
              # counter 0..71) chosen to meet each consumer's deadline while
              # spreading PE work into the ACT-limited stretches.
              def qk(t, sp, mi):
                  return lambda: qk_proj(t, sp, mi)

              def vp(b):
                  return lambda: v_proj(b)

              sched = {
                  0: [vp(0)], 1: [vp(1)], 2: [vp(2)], 3: [vp(3)],
                  4: [vp(4)], 5: [vp(5)], 6: [vp(6)], 7: [vp(7)],
                  10: [qk("q", 2, 0)], 12: [qk("q", 2, 1)],
                  14: [qk("q", 3, 0)], 16: [qk("q", 3, 1)],
                  18: [qk("k", 1, 1)],
                  26: [qk("k", 2, 0)], 28: [vp(8)], 30: [vp(9)],
                  33: [qk("k", 3, 0)], 34: [vp(10)], 35: [vp(11)],
                  36: [vp(12)], 37: [vp(13)], 38: [vp(14)], 39: [vp(15)],
                  56: [qk("k", 2, 1)], 60: [qk("k", 3, 1)],
              }
              # g0 outproj units get slotted into g1 once ctn_g0 exists
              OUTPROJ_SLOTS = (40, 42, 44, 46, 48, 50, 52, 54, 57, 58, 61, 62)

              def outproj_unit(ctn, g, jc, piece, alt=False):
                  # alternate the PSUM ring (tail only — sT ring is live
                  # during attention) and the PSUM->SBUF copy engine so the
                  # drain isn't serialized on one resource
                  if alt and (jc + piece) % 2:
                      po = ps_sT.tile([128, 512], f32, tag="sT", name=f"po_{jc}_{piece}")
                  else:
                      po = ps_mm.tile([128, 512], f32, tag="mm")
                  nc.tensor.matmul(
                      po[:],
                      wo_sb[:, 0, 128 * jc : 128 * jc + 128],
                      ctn[piece][:, 0, :],
                      start=True,
                      stop=False,
                  )
                  nc.tensor.matmul(
                      po[:],
                      wo_sb[0:64, 1, 128 * jc : 128 * jc + 128],
                      ctn[piece][0:64, 1, :],
                      start=False,
                      stop=True,
                  )
                  ot = wrk.tile([128, 512], bf16, tag="ot", bufs=8)
                  if alt and (jc + piece) % 2:
                      # tail only: ACT is idle once the exps are done
                      nc.scalar.copy(ot[:], po[:])
                  else:
                      nc.vector.tensor_copy(ot[:], po[:])
                  dma_eng = nc.scalar if (jc + piece) % 2 else nc.sync
                  dma_eng.dma_start(
                      outT_d[
                          128 * jc : 128 * jc + 128,
                          QS * g + 512 * piece : QS * g + 512 * piece + 512,
                      ],
                      ot[:],
                  )

              # ---- attention + out-projection per q superblock ----
              slot = 0
              for g in range(NG):
                  # normalized ctxT per 512-piece, bf16
                  # packed: [0:64,0]=h0, [64:128,0]=h1, [0:64,1]=h2
                  ctn = [
                      nrm.tile([128, 2, 512], bf16, tag=f"ctn{p}", name=f"ctn{p}_{g}")
                      for p in range(2)
                  ]
                  for h in range(NH):
                      ctx = [
                          ps_ctx.tile(
                              [128, 512], f32, tag=f"ctx{p}", name=f"ctx{p}_{g}_{h}"
                          )
                          for p in range(2)
                      ]
                      kpairs = 4 * (g + 1)

                      def emit_pv(pt_, pb_, i_, g_=g, h_=h, ctx_=None, ctn_=None):
                          # PV: fp8 DoubleRow over pair i_ (emitted one pair
                          # late so it never waits on the pair's exp).
                          # One start-chain per ctx bank: the bf16 q<128 patch
                          # is the bank's opening write in (g0, pair0, piece0).
                          if g_ == 0 and i_ == 0:
                              nc.tensor.matmul(
                                  ctx_[0][:, 0:128], v0b_sb[:, h_, :], pb_[:],
                                  start=True, stop=False,
                                  skip_group_check=True,
                              )
                          if DEBUG and g_ == 0 and h_ == 0 and i_ == 0:
                              nc.sync.dma_start(dbg["dpt0"][:], pt_[:])
                          pq0_ = max(0, 128 * (2 * i_ - 8 * g_))
                          for piece in range(2):
                              lp = 2 * piece + 1 + 4 * g_
                              lo = max(pq0_, 512 * piece)
                              patch0 = g_ == 0 and piece == 0
                              if patch0 and i_ == 0:
                                  lo = 128  # q<128 handled by the bf16 patch
                              hi = 512 * piece + 512
                              if lo >= hi:
                                  continue
                              nc.tensor.matmul(
                                  ctx_[piece][:, lo - 512 * piece : hi - 512 * piece],
                                  v8_sb[i_][:, :, h_, :],
                                  pt_[:, :, lo:hi],
                                  start=(i_ == 0 and not patch0),
                                  stop=(i_ == lp),
                                  perf_mode=DR,
                                  skip_group_check=True,
                              )
                          # normalization stage A (DVE/Pool reciprocal
                          # chain) per piece as soon as it stops; stage B
                          # (PE broadcast + multiply) is deferred one pair so
                          # PE never waits on the chain
                          for piece in range(2):
                              if i_ != 2 * piece + 1 + 4 * g_:
                                  continue
                              rec = nrm.tile([1, 512], f32, tag="rec")
                              nc.vector.reciprocal_approx_fast(
                                  out=rec[:], in_=ctx_[piece][0:1, :]
                              )
                              if DEBUG and g_ == 0 and h_ == 0:
                                  nc.sync.dma_start(dbg[f"drec{piece}"][:], rec[:])
                              recb = nrm.tile([1, 512], bf16, tag="recb")
                              nc.gpsimd.tensor_copy(recb[:], rec[:])

                              def norm_b(piece=piece, recb=recb, g_=g_, h_=h_,
                                         ctx_=ctx_, ctn_=ctn_):
                                  bc = ps_mm.tile([64, 512], f32, tag="mm")
                                  nc.tensor.matmul(
                                      bc[:], ones_b[:, 0:64], recb[:],
                                      start=True, stop=True,
                                  )
                                  cts = nrm.tile([64, 512], bf16, tag="cts")
                                  nc.vector.tensor_copy(
                                      cts[:], ctx_[piece][64:128, :]
                                  )
                                  dst = (
                                      ctn_[piece][64 * h_ : 64 * h_ + 64, 0, :]
                                      if h_ < 2
                                      else ctn_[piece][0:64, 1, :]
                                  )
                                  nc.vector.tensor_mul(dst, cts[:], bc[:])

                              pending_b_new.append(norm_b)

                      prev = None  # (pt, pt0b, pair) awaiting PV emission
                      pt0b = None
                      pending_b = []      # norm stage-B ready to emit
                      pending_b_new = []  # norm stage-B queued one extra pair
                      for i in range(kpairs):
                          pq0 = max(0, 128 * (2 * i - 8 * g))  # pair's first valid col
                          pt = wrk.tile([128, 2, QS], fp8, tag="pt")
                          for par in range(2):
                              c = 2 * i + par
                              j = c - 8 * g
                              q0 = max(0, 128 * j)
                              sT = ps_sT.tile([128, QS], f32, tag="sT")
                              # PSUM accumulation is tracked per 2KB bank:
                              # exactly ONE start=True (first write) and one
                              # stop=True (last write) per 512-col piece, and
                              # every byte exp reads must be written after the
                              # start (pending-zero bytes read back garbage).
                              for piece in range(2):
                                  p0, p1 = 512 * piece, 512 * piece + 512
                                  segs = []  # (kind, lo, hi)
                                  if par == 1 and q0 > pq0:
                                      # odd chunk's leading masked cols: rank-1
                                      # -1e9 fill so the pair-exp writes zeros
                                      nl, nh = max(pq0, p0), min(q0, p1)
                                      if nl < nh:
                                          segs.append(("neg", nl, nh))
                                  if j >= 0 and p0 <= q0 < p1:
                                      segs.append(("diag", q0, q0 + 128))
                                      if q0 + 128 < p1:
                                          segs.append(("sc", q0 + 128, p1))
                                  else:
                                      lo = max(q0, p0)
                                      if lo < p1:
                                          segs.append(("sc", lo, p1))
                                  # start only on the bank's first write; stop
                                  # on each byte-range's final write
                                  for si, (kind, lo, hi) in enumerate(segs):
                                      st = si == 0
                                      if kind == "neg":
                                          nc.tensor.matmul(
                                              sT[:, lo:hi], ones_b[:], negrow_b[:],
                                              start=st, stop=True,
                                              skip_group_check=True,
                                          )
                                      elif kind == "diag":
                                          nc.tensor.matmul(
                                              sT[:, lo:hi],
                                              head_ap("k", h, 128 * c, 128 * c + 128),
                                              head_ap("q", h, QS * g + lo, QS * g + hi),
                                              start=st, stop=False,
                                              skip_group_check=True,
                                          )
                                          nc.tensor.matmul(
                                              sT[:, lo:hi],
                                              ident_sb[:], maskneg_sb[:],
                                              start=False, stop=True,
                                              skip_group_check=True,
                                          )
                                      else:
                                          nc.tensor.matmul(
                                              sT[:, lo:hi],
                                              head_ap("k", h, 128 * c, 128 * c + 128),
                                              head_ap("q", h, QS * g + lo, QS * g + hi),
                                              start=st, stop=True,
                                              skip_group_check=True,
                                          )
                              nc.scalar.activation(
                                  pt[:, par, pq0:QS], sT[:, pq0:QS], Exp,
                                  scale=float(SCALE),
                              )
                              if g == 0 and i == 0 and par == 0:
                                  # bf16 patch of P for q<128
                                  pt0b = wrk.tile([128, 128], bf16, tag="pt0b")
                                  nc.scalar.activation(
                                      pt0b[:], sT[:, 0:128], Exp, scale=float(SCALE)
                                  )
                                  if DEBUG and h == 0:
                                      nc.sync.dma_start(dbg["dpt0b"][:], pt0b[:])
                              if par == 0:
                                  for f in pending_b:
                                      f()
                                  pending_b = pending_b_new
                                  pending_b_new = []
                                  if prev is not None:
                                      emit_pv(*prev, ctx_=ctx, ctn_=ctn)
                              for f in sched.pop(slot, ()):
                                  f()
                              slot += 1
                          prev = (pt, pt0b, i)
                      emit_pv(*prev, ctx_=ctx, ctn_=ctn)
                      for f in pending_b + pending_b_new:
                          f()
                  if DEBUG and g == 0:
                      nc.sync.dma_start(dbg["dctn0"][:], ctn[0][:])
                      nc.sync.dma_start(dbg["dctn1"][:], ctn[1][:])
                      nc.sync.dma_start(dbg["dv8_0"][:], v8_sb[0][:])
                      nc.sync.dma_start(dbg["dv0b"][:], v0b_sb[:])
                      nc.sync.dma_start(dbg["dq0"][:], qk_sb["q"][0][:])
                      nc.sync.dma_start(dbg["dk0"][:], qk_sb["k"][0][:])
                  # out projection: g0's units are slotted into g1's
                  # attention as fillers; g1's run inline at the end.
                  if g == 0:
                      units = [
                          (jc, piece) for jc in range(6) for piece in range(2)
                      ]
                      for s, (jc, piece) in zip(OUTPROJ_SLOTS, units):
                          sched.setdefault(s, []).append(
                              lambda c_=ctn, jc_=jc, p_=piece: outproj_unit(
                                  c_, 0, jc_, p_
                              )
                          )
                  else:
                      # piece 0 first: its norms are already done, so PE
                      # stays busy while piece 1's norm chain drains on DVE
                      for piece in range(2):
                          for jc in range(6):
                              outproj_unit(ctn, g, jc, piece, alt=True)

    nc.compile()
    _cache[key] = nc
    return nc


def kernel(x, Wq, bq, Wk, bk, Wv, bv, Wo, bo):
    out, _ = run(x, Wq, bq, Wk, bk, Wv, bv, Wo, bo)
    return out


def build_in_maps(x, Wq, bq, Wk, bk, Wv, bv, Wo, bo=None):
    bf = ml_dtypes.bfloat16
    f8 = ml_dtypes.float8_e4m3
    x = np.asarray(x, np.float32)
    Wq, bq = np.asarray(Wq, np.float32), np.asarray(bq, np.float32)
    Wk, bk = np.asarray(Wk, np.float32), np.asarray(bk, np.float32)
    Wv, bv = np.asarray(Wv, np.float32), np.asarray(bv, np.float32)
    Wo = np.asarray(Wo, np.float32)

    # additive causal mask for a diagonal 128-block: 0 where q >= k, -1e9 else
    maskneg = np.where(
        np.tri(128, 128, 0, dtype=bool).T, np.float32(0.0), np.float32(NEG)
    ).astype(bf)
    # note: tri().T gives [k, q] upper-tri (q >= k -> valid -> 0)
    ident = np.eye(128, dtype=np.float32).astype(bf)

    in_maps = []
    for c in range(NCORES):
        b, rs = c // 4, (c % 4) * NH * HD
        re = rs + NH * HD
        woP = np.zeros((128, 2, D), np.float32)
        woP[:, 0, :] = Wo[:, rs : rs + 128].T
        woP[0:64, 1, :] = Wo[:, rs + 128 : rs + 192].T
        wv8 = np.zeros((D, 384), np.float32)
        bv_row = np.zeros((1, 384), np.float32)
        for h in range(NH):
            wv8[:, 128 * h + 64 : 128 * h + 128] = Wv[rs + 64 * h : rs + 64 * h + 64].T
            bv_row[0, 128 * h + 64 : 128 * h + 128] = bv[rs + 64 * h : rs + 64 * h + 64]
            bv_row[0, 128 * h] = 1.0
        xT = np.ascontiguousarray(x[b].T)
        in_maps.append(
            {
                "xT": xT.astype(bf),
                "x8T": xT.astype(f8),
                "wqT": np.ascontiguousarray(Wq[rs:re].T).astype(bf),
                "wkT": np.ascontiguousarray(Wk[rs:re].T).astype(bf),
                "wv8T": wv8.astype(f8),
                "wvT": wv8.astype(bf),
                "woT": woP.astype(bf),
                "bq01": bq[rs : rs + 128].reshape(128, 1).copy(),
                "bq2": bq[rs + 128 : re].reshape(64, 1).copy(),
                "bk01": bk[rs : rs + 128].reshape(128, 1).copy(),
                "bk2": bk[rs + 128 : re].reshape(64, 1).copy(),
                "bv": bv_row.astype(bf),
                "maskneg": maskneg,
                "ident": ident,
            }
        )
    return in_maps


def run(x, Wq, bq, Wk, bk, Wv, bv, Wo, bo, trace=False):
    from concourse.bass_utils import run_bass_kernel_spmd

    nc = _build()
    bo = np.asarray(bo, np.float32)
    in_maps = build_in_maps(x, Wq, bq, Wk, bk, Wv, bv, Wo)
    res = run_bass_kernel_spmd(nc, in_maps, list(range(NCORES)), trace=trace)
    out = np.zeros((B, S, D), np.float32)
    for b in range(B):
        acc = np.zeros((D, S), np.float32)
        for c in range(4 * b, 4 * b + 4):
            acc += res.results[c]["outT"].astype(np.float32)
        out[b] = acc.T + bo
    return out, res


# revision 56
# speedup vs baseline: 1.0720x; 1.0720x over previous
"""Causal multi-head attention (B=2, S=2048, D=768, H=12) on 8 TRN2 NeuronCores.

Sharding: core c handles batch c//4, heads 3*(c%4) .. 3*(c%4)+3.

Per core (bf16 operands, fp8e4m3 DoubleRow PV/V-proj, fp32 PSUM):
  - q/k projections bf16, chunk-major across 6 parallel PSUM groups at
    startup so PE rides the serial input-DMA stream.
  - v projection fp8 DoubleRow (256-deep contraction, 0.5 cyc/row); per-head
    128-col groups laid out [ones | 63 pad | v(64)] so the denominator lands
    on ctx partition 0 (reciprocal_approx_fast silently misreads PSUM at
    nonzero partition offsets) and v sits 64-aligned (>32-partition reads
    must be 0/64-aligned). Blocks 0-1 run bf16 (block 0 feeds the q<128
    patch; block 1 hides the x8 DMA arrival).
  - scores TRANSPOSED sT[k,q] = K.Q^T bf16; causal mask ACCUMULATED into the
    diagonal 128-block via identity-lhsT matmul; odd chunks of a k-pair get
    a rank-1 -1e9 fill so the pair-exp writes exact zeros. Exactly one
    start=True per PSUM bank (start zeroes the whole 2KB region) and
    stop=True on each byte-range's final write.
  - exp on ACT straight from PSUM -> P^T fp8 in [128,2,QS] pair tiles; PV is
    fp8 DoubleRow over k-pairs, emitted one pair late so it never waits on
    exp. q<128 is patched with a bf16 exp + bf16 PV (fp8 softmax-weight
    error spikes at tiny Neff). Remaining projections/outproj work is
    slot-scheduled into the attention loop as PE fillers.
  - normalization: DVE reciprocal_approx_fast reads the denominator row from
    PSUM; bf16 copy on GpSimd; ones-matmul broadcast; DVE multiply -> bf16
    ctn. The PE-visible stage is deferred two pairs to hide the chain.
  - out-projection bf16; g0's units interleave into g1 as fillers; outputs
    stored bf16 on alternating HWDGE queues, summed across cores on host.
"""

import numpy as np
import ml_dtypes

B, S, D, H, HD = 2, 2048, 768, 12, 64
NH = 3                      # heads per core
NCORES = 8
SCALE = 1.0 / np.sqrt(HD)
QS = 1024                   # q superblock width
NG = S // QS                # 2 q superblocks
NKC = S // 128              # 16 k chunks
NXC = D // 128              # 6 contraction chunks of 128 over D
NEG = -1.0e9                # causal mask fill (exp(NEG*SCALE) == 0)

_cache = {}
DEBUG = False


def _enable_ldw_opt():
    """No-op: walrus's ldw-opt rejects the explicit InstLdweights that
    bf16/fp8 matmuls emit (it only composes with f32r's implicit loads)."""


def _build(reps=1):
    _enable_ldw_opt()
    key = ("nc", reps)
    if key in _cache:
        return _cache[key]
    import concourse.bacc as bacc
    import concourse.mybir as mybir
    import concourse.tile as tile

    f32 = mybir.dt.float32
    bf16 = mybir.dt.bfloat16
    fp8 = mybir.dt.float8e4
    DR = mybir.MatmulPerfMode.DoubleRow
    Exp = mybir.ActivationFunctionType.Exp
    add_op = mybir.AluOpType.add

    nc = bacc.Bacc(None, target_bir_lowering=False, debug=False, num_devices=NCORES)

    xT_d = nc.dram_tensor("xT", [D, S], bf16, kind="ExternalInput")
    x8T_d = nc.dram_tensor("x8T", [D, S], fp8, kind="ExternalInput")
    wqT_d = nc.dram_tensor("wqT", [D, NH * HD], bf16, kind="ExternalInput")
    wkT_d = nc.dram_tensor("wkT", [D, NH * HD], bf16, kind="ExternalInput")
    wv8T_d = nc.dram_tensor("wv8T", [D, 384], fp8, kind="ExternalInput")
    wvT_d = nc.dram_tensor("wvT", [D, 384], bf16, kind="ExternalInput")
    woT_d = nc.dram_tensor("woT", [128, 2, D], bf16, kind="ExternalInput")
    bq01_d = nc.dram_tensor("bq01", [128, 1], f32, kind="ExternalInput")
    bq2_d = nc.dram_tensor("bq2", [64, 1], f32, kind="ExternalInput")
    bk01_d = nc.dram_tensor("bk01", [128, 1], f32, kind="ExternalInput")
    bk2_d = nc.dram_tensor("bk2", [64, 1], f32, kind="ExternalInput")
    bv_d = nc.dram_tensor("bv", [1, 384], bf16, kind="ExternalInput")
    maskneg_d = nc.dram_tensor("maskneg", [128, 128], bf16, kind="ExternalInput")
    ident_d = nc.dram_tensor("ident", [128, 128], bf16, kind="ExternalInput")
    outT_d = nc.dram_tensor("outT", [D, S], bf16, kind="ExternalOutput")
    dbg = {}
    if DEBUG:
        import concourse.mybir as _mb
        dbg["dpt0"] = nc.dram_tensor("dpt0", [128, 2, QS], fp8, kind="ExternalOutput")
        dbg["dv8_0"] = nc.dram_tensor("dv8_0", [128, 2, NH, 128], fp8, kind="ExternalOutput")
        dbg["dv0b"] = nc.dram_tensor("dv0b", [128, NH, 128], bf16, kind="ExternalOutput")
        dbg["drec0"] = nc.dram_tensor("drec0", [1, 512], f32, kind="ExternalOutput")
        dbg["drec1"] = nc.dram_tensor("drec1", [1, 512], f32, kind="ExternalOutput")
        dbg["dq0"] = nc.dram_tensor("dq0", [128, 2, 512], bf16, kind="ExternalOutput")
        dbg["dk0"] = nc.dram_tensor("dk0", [128, 2, 512], bf16, kind="ExternalOutput")
        dbg["dctn0"] = nc.dram_tensor("dctn0", [128, 2, 512], bf16, kind="ExternalOutput")
        dbg["dcts0"] = nc.dram_tensor("dcts0", [64, 512], bf16, kind="ExternalOutput")
        dbg["dden0"] = nc.dram_tensor("dden0", [1, 512], f32, kind="ExternalOutput")
        dbg["dpt0b"] = nc.dram_tensor("dpt0b", [128, 128], bf16, kind="ExternalOutput")
        dbg["dctn1"] = nc.dram_tensor("dctn1", [128, 2, 512], bf16, kind="ExternalOutput")

    with tile.TileContext(nc) as tc:
        with (
            tc.tile_pool(name="const", bufs=1) as cst,
            tc.tile_pool(name="work", bufs=3) as wrk,
            tc.tile_pool(name="norm", bufs=2) as nrm,
            tc.tile_pool(name="ps_sT", bufs=2, space="PSUM") as ps_sT,
            tc.tile_pool(name="ps_ctx", bufs=1, space="PSUM") as ps_ctx,
            tc.tile_pool(name="ps_mm", bufs=2, space="PSUM") as ps_mm,
        ):
         for _rep in range(reps):
              # ---- constant / persistent SBUF ----
              # Small consts on the gpsimd SWDGE queue (its ~1.1us/DMA serial
              # generation cost is fine for these); weights + x interleaved
              # across the two HWDGE queues (SP, ACT) ordered by first use.
              maskneg_sb = cst.tile([128, 128], bf16)
              nc.gpsimd.dma_start(maskneg_sb[:], maskneg_d[:])
              ident_sb = cst.tile([128, 128], bf16)
              nc.gpsimd.dma_start(ident_sb[:], ident_d[:])
              bq01 = cst.tile([128, 1], f32)
              nc.gpsimd.dma_start(bq01[:], bq01_d[:])
              bq2 = cst.tile([64, 1], f32)
              nc.gpsimd.dma_start(bq2[:], bq2_d[:])
              bk01 = cst.tile([128, 1], f32)
              nc.gpsimd.dma_start(bk01[:], bk01_d[:])
              bk2 = cst.tile([64, 1], f32)
              nc.gpsimd.dma_start(bk2[:], bk2_d[:])
              bv_sb = cst.tile([1, 384], bf16)
              nc.gpsimd.dma_start(bv_sb[:], bv_d[:])

              # DMA engines serialize at HBM bandwidth, so submission order ==
              # arrival order: weights first, then x chunk-by-chunk (the
              # chunk-major initial projections ride this stream), then the
              # late-use tensors (wv bf16, wo, x8, wv8).
              xT_r = xT_d[:].rearrange("(c p) s -> p c s", p=128)
              x8_r = x8T_d[:].rearrange("(c p) s -> p c s", p=128)
              wq_sb = cst.tile([128, NXC, NH * HD], bf16)
              nc.sync.dma_start(wq_sb[:], wqT_d[:].rearrange("(c p) m -> p c m", p=128))
              wk_sb = cst.tile([128, NXC, NH * HD], bf16)
              nc.scalar.dma_start(wk_sb[:], wkT_d[:].rearrange("(c p) m -> p c m", p=128))
              x_sb = []
              for c in range(NXC):
                  xc = cst.tile([128, S], bf16, tag=f"x{c}")
                  eng = nc.sync if c % 2 == 0 else nc.scalar
                  eng.dma_start(xc[:], xT_r[:, c, :])
                  x_sb.append(xc)
              wv_sb = cst.tile([128, NXC, 384], bf16)
              nc.scalar.dma_start(wv_sb[:], wvT_d[:].rearrange("(c p) m -> p c m", p=128))
              x8_sb = cst.tile([128, NXC, S], fp8)
              nc.sync.dma_start(x8_sb[:, 0:3, :], x8_r[:, 0:3, :])
              nc.scalar.dma_start(x8_sb[:, 3:6, :], x8_r[:, 3:6, :])
              wv8_sb = cst.tile([128, NXC, 384], fp8)
              nc.sync.dma_start(wv8_sb[:], wv8T_d[:].rearrange("(c p) m -> p c m", p=128))
              wo_sb = cst.tile([128, 2, D], bf16)
              nc.scalar.dma_start(wo_sb[:], woT_d[:])

              ones_f = cst.tile([1, 128], f32)
              nc.vector.memset(ones_f[:], 1.0)
              ones_b = cst.tile([1, 128], bf16)
              nc.vector.tensor_copy(ones_b[:], ones_f[:])
              ones_r = cst.tile([1, 128], mybir.dt.float32r)
              nc.vector.tensor_copy(ones_r[:], ones_f[:])
              negrow_f = cst.tile([1, 128], f32)
              nc.vector.memset(negrow_f[:], NEG)
              negrow_b = cst.tile([1, 128], bf16)
              nc.vector.tensor_copy(negrow_b[:], negrow_f[:])

              # persistent activations:
              # q/k: one tile per 512-wide s-super; slot 0 holds heads 0/1
              # stacked on partitions, slot 1 head 2.
              qk_sb = {
                  t: [
                      cst.tile(
                          [128, 2, 512], bf16, tag=f"{t}sp{sp}", name=f"{t}sp{sp}"
                      )
                      for sp in range(4)
                  ]
                  for t in ("q", "k")
              }
              # v_aug fp8 per k-pair: [128, 2, NH, 65]; col 64 of each head
              # group is the ones column (from the bias matmul's bv layout).
              v8_sb = [
                  cst.tile([128, 2, NH, 128], fp8, tag=f"vp{i}", name=f"vp{i}")
                  for i in range(NKC // 2)
              ]
              # bf16 copy of v block 0 for the q<128 patch
              v0b_sb = cst.tile([128, NH, 128], bf16)

              def head_ap(t, h, lo, hi):
                  """AP for head h, global columns [lo, hi) (within one super)."""
                  sp, o = lo // 512, lo % 512
                  tile_ = qk_sb[t][sp]
                  if h < 2:
                      return tile_[64 * h : 64 * h + 64, 0, o : o + hi - lo]
                  return tile_[0:64, 1, o : o + hi - lo]

              def qk_proj(t, sp, mi):
                  scols = slice(512 * sp, 512 * sp + 512)
                  m0, msz, slot = ((0, 128, 0), (128, 64, 1))[mi]
                  b01, b2 = (bq01, bq2) if t == "q" else (bk01, bk2)
                  p = ps_mm.tile([128, 512], f32, tag="mm")
                  for c in range(NXC):
                      nc.tensor.matmul(
                          p[:msz, :],
                          (wq_sb if t == "q" else wk_sb)[:, c, m0 : m0 + msz],
                          x_sb[c][:, scols],
                          start=(c == 0),
                          stop=(c == NXC - 1),
                      )
                  nc.vector.tensor_scalar(
                      out=qk_sb[t][sp][:msz, slot, :],
                      in0=p[:msz, :],
                      scalar1=(b01 if mi == 0 else b2)[:msz],
                      scalar2=None,
                      op0=add_op,
                  )

              def v_proj(blk):
                  # fp8 DoubleRow over 3 contraction pairs; the bf16 rank-1
                  # bias matmul adds bv + the ones column of each head group.
                  # Block 0 runs bf16: it feeds the q<128 patch, whose whole
                  # point is bf16-quality v for the tiny-Neff positions.
                  p = ps_mm.tile([128, 384], f32, tag="mm")
                  if blk < 2:
                      for c in range(NXC):
                          nc.tensor.matmul(
                              p[:],
                              x_sb[c][:, 128 * blk : 128 * blk + 128],
                              wv_sb[:, c, :],
                              start=(c == 0),
                              stop=False,
                          )
                  else:
                      for i in range(3):
                          nc.tensor.matmul(
                              p[:],
                              x8_sb[:, 2 * i : 2 * i + 2, 128 * blk : 128 * blk + 128],
                              wv8_sb[:, 2 * i : 2 * i + 2, :],
                              start=(i == 0),
                              stop=False,
                              perf_mode=DR,
                          )
                  nc.tensor.matmul(p[:], ones_b[:], bv_sb[:], start=False, stop=True)
                  nc.vector.tensor_copy(
                      v8_sb[blk // 2][:, blk % 2, :, :], p[:, 0 : NH * 128]
                  )
                  if blk == 0:
                      nc.vector.tensor_copy(v0b_sb[:], p[:, 0 : NH * 128])

              # ---- initial projections: just enough for g=0 h=0 ----
              # Chunk-major across 6 parallel PSUM accumulation groups
              # (borrowing slots from every pool) so PE consumes each x chunk
              # the moment it lands instead of stalling group-by-group.
              igroups = [
                  ("q", 0, 0), ("q", 1, 0), ("k", 0, 0),
                  ("q", 1, 1), ("q", 0, 1), ("k", 0, 1),
              ]
              ipsum = [
                  ps_mm.tile([128, 512], f32, tag="mm", name="ip0"),
                  ps_mm.tile([128, 512], f32, tag="mm", name="ip1"),
                  ps_sT.tile([128, QS], f32, tag="sT", name="ip2"),
                  ps_sT.tile([128, QS], f32, tag="sT", name="ip3"),
                  ps_ctx.tile([128, 512], f32, tag="ctx0", name="ip4"),
                  ps_ctx.tile([128, 512], f32, tag="ctx1", name="ip5"),
              ]
              for c in range(NXC):
                  for gi, (t, sp, mi) in enumerate(igroups):
                      m0, msz, _ = ((0, 128, 0), (128, 64, 1))[mi]
                      nc.tensor.matmul(
                          ipsum[gi][:msz, 0:512],
                          (wq_sb if t == "q" else wk_sb)[:, c, m0 : m0 + msz],
                          x_sb[c][:, 512 * sp : 512 * sp + 512],
                          start=(c == 0),
                          stop=(c == NXC - 1),
                      )
              for gi, (t, sp, mi) in enumerate(igroups):
                  m0, msz, slot = ((0, 128, 0), (128, 64, 1))[mi]
                  b01, b2 = (bq01, bq2) if t == "q" else (bk01, bk2)
                  nc.vector.tensor_scalar(
                      out=qk_sb[t][sp][:msz, slot, :],
                      in0=ipsum[gi][:msz, 0:512],
                      scalar1=(b01 if mi == 0 else b2)[:msz],
                      scalar2=None,
                      op0=add_op,
                  )
              qk_proj("k", 1, 0)

              # remaining projection + g0-outproj work, interleaved into the
              # attention loop at fixed chunk slots (slot = global chunk
              # counter 0..71) chosen to meet each consumer's deadline while
              # spreading PE work into the ACT-limited stretches.
              def qk(t, sp, mi):
                  return lambda: qk_proj(t, sp, mi)

              def vp(b):
                  return lambda: v_proj(b)

              sched = {
                  0: [vp(0)], 1: [vp(1)], 2: [vp(2)], 3: [vp(3)],
                  4: [vp(4)], 5: [vp(5)], 6: [vp(6)], 7: [vp(7)],
                  10: [qk("q", 2, 0)], 12: [qk("q", 2, 1)],
                  14: [qk("q", 3, 0)], 16: [qk("q", 3, 1)],
                  18: [qk("k", 1, 1)],
                  26: [qk("k", 2, 0)], 28: [vp(8)], 30: [vp(9)],
                  33: [qk("k", 3, 0)], 34: [vp(10)], 35: [vp(11)],
                  36: [vp(12)], 37: [vp(13)], 38: [vp(14)], 39: [vp(15)],
                  56: [qk("k", 2, 1)], 60: [qk("k", 3, 1)],
              }
              # g0 outproj units get slotted into g1 once ctn_g0 exists
              OUTPROJ_SLOTS = (40, 42, 44, 46, 48, 50, 52, 54, 57, 58, 61, 62)

              def outproj_unit(ctn, g, jc, piece, alt=False):
                  # alternate the PSUM ring (tail only — sT ring is live
                  # during attention) and the PSUM->SBUF copy engine so the
                  # drain isn't serialized on one resource
                  if alt and (jc + piece) % 2:
                      po = ps_sT.tile([128, 512], f32, tag="sT", name=f"po_{jc}_{piece}")
                  else:
                      po = ps_mm.tile([128, 512], f32, tag="mm")
                  nc.tensor.matmul(
                      po[:],
                      wo_sb[:, 0, 128 * jc : 128 * jc + 128],
                      ctn[piece][:, 0, :],
                      start=True,
                      stop=False,
                  )
                  nc.tensor.matmul(
                      po[:],
                      wo_sb[0:64, 1, 128 * jc : 128 * jc + 128],
                      ctn[piece][0:64, 1, :],
                      start=False,
                      stop=True,
                  )
                  ot = wrk.tile([128, 512], bf16, tag="ot", bufs=8)
                  if alt and (jc + piece) % 2:
                      # tail only: ACT is idle once the exps are done
                      nc.scalar.copy(ot[:], po[:])
                  else:
                      nc.vector.tensor_copy(ot[:], po[:])
                  dma_eng = nc.scalar if (jc + piece) % 2 else nc.sync
                  dma_eng.dma_start(
                      outT_d[
                          128 * jc : 128 * jc + 128,
                          QS * g + 512 * piece : QS * g + 512 * piece + 512,
                      ],
                      ot[:],
                  )

              # ---- attention + out-projection per q superblock ----
              slot = 0
              for g in range(NG):
                  # normalized ctxT per 512-piece, bf16
                  # packed: [0:64,0]=h0, [64:128,0]=h1, [0:64,1]=h2
                  ctn = [
                      nrm.tile([128, 2, 512], bf16, tag=f"ctn{p}", name=f"ctn{p}_{g}")
                      for p in range(2)
                  ]
                  for h in range(NH):
                      ctx = [
                          ps_ctx.tile(
                              [128, 512], f32, tag=f"ctx{p}", name=f"ctx{p}_{g}_{h}"
                          )
                          for p in range(2)
                      ]
                      kpairs = 4 * (g + 1)

                      def emit_pv(pt_, pb_, i_, g_=g, h_=h, ctx_=None, ctn_=None):
                          # PV: fp8 DoubleRow over pair i_ (emitted one pair
                          # late so it never waits on the pair's exp).
                          # One start-chain per ctx bank: the bf16 q<128 patch
                          # is the bank's opening write in (g0, pair0, piece0).
                          if g_ == 0 and i_ == 0:
                              nc.tensor.matmul(
                                  ctx_[0][:, 0:128], v0b_sb[:, h_, :], pb_[:],
                                  start=True, stop=False,
                                  skip_group_check=True,
                              )
                          if DEBUG and g_ == 0 and h_ == 0 and i_ == 0:
                              nc.sync.dma_start(dbg["dpt0"][:], pt_[:])
                          pq0_ = max(0, 128 * (2 * i_ - 8 * g_))
                          for piece in range(2):
                              lp = 2 * piece + 1 + 4 * g_
                              lo = max(pq0_, 512 * piece)
                              patch0 = g_ == 0 and piece == 0
                              if patch0 and i_ == 0:
                                  lo = 128  # q<128 handled by the bf16 patch
                              hi = 512 * piece + 512
                              if lo >= hi:
                                  continue
                              nc.tensor.matmul(
                                  ctx_[piece][:, lo - 512 * piece : hi - 512 * piece],
                                  v8_sb[i_][:, :, h_, :],
                                  pt_[:, :, lo:hi],
                                  start=(i_ == 0 and not patch0),
                                  stop=(i_ == lp),
                                  perf_mode=DR,
                                  skip_group_check=True,
                              )
                          # normalization stage A (DVE/Pool reciprocal
                          # chain) per piece as soon as it stops; stage B
                          # (PE broadcast + multiply) is deferred one pair so
                          # PE never waits on the chain
                          for piece in range(2):
                              if i_ != 2 * piece + 1 + 4 * g_:
                                  continue
                              rec = nrm.tile([1, 512], f32, tag="rec")
                              nc.vector.reciprocal_approx_fast(
                                  out=rec[:], in_=ctx_[piece][0:1, :]
                              )
                              if DEBUG and g_ == 0 and h_ == 0:
                                  nc.sync.dma_start(dbg[f"drec{piece}"][:], rec[:])
                              recb = nrm.tile([1, 512], bf16, tag="recb")
                              nc.vector.tensor_copy(recb[:], rec[:])

                              def norm_b(piece=piece, recb=recb, g_=g_, h_=h_,
                                         ctx_=ctx_, ctn_=ctn_):
                                  bc = ps_mm.tile([64, 512], f32, tag="mm")
                                  nc.tensor.matmul(
                                      bc[:], ones_b[:, 0:64], recb[:],
                                      start=True, stop=True,
                                  )
                                  cts = nrm.tile([64, 512], bf16, tag="cts")
                                  nc.vector.tensor_copy(
                                      cts[:], ctx_[piece][64:128, :]
                                  )
                                  dst = (
                                      ctn_[piece][64 * h_ : 64 * h_ + 64, 0, :]
                                      if h_ < 2
                                      else ctn_[piece][0:64, 1, :]
                                  )
                                  nc.vector.tensor_mul(dst, cts[:], bc[:])

                              pending_b_new.append(norm_b)

                      prev = None  # (pt, pt0b, pair) awaiting PV emission
                      pt0b = None
                      pending_b = []      # norm stage-B ready to emit
                      pending_b_new = []  # norm stage-B queued one extra pair
                      for i in range(kpairs):
                          pq0 = max(0, 128 * (2 * i - 8 * g))  # pair's first valid col
                          pt = wrk.tile([128, 2, QS], fp8, tag="pt")
                          for par in range(2):
                              c = 2 * i + par
                              j = c - 8 * g
                              q0 = max(0, 128 * j)
                              sT = ps_sT.tile([128, QS], f32, tag="sT")
                              # PSUM accumulation is tracked per 2KB bank:
                              # exactly ONE start=True (first write) and one
                              # stop=True (last write) per 512-col piece, and
                              # every byte exp reads must be written after the
                              # start (pending-zero bytes read back garbage).
                              for piece in range(2):
                                  p0, p1 = 512 * piece, 512 * piece + 512
                                  segs = []  # (kind, lo, hi)
                                  if par == 1 and q0 > pq0:
                                      # odd chunk's leading masked cols: rank-1
                                      # -1e9 fill so the pair-exp writes zeros
                                      nl, nh = max(pq0, p0), min(q0, p1)
                                      if nl < nh:
                                          segs.append(("neg", nl, nh))
                                  if j >= 0 and p0 <= q0 < p1:
                                      segs.append(("diag", q0, q0 + 128))
                                      if q0 + 128 < p1:
                                          segs.append(("sc", q0 + 128, p1))
                                  else:
                                      lo = max(q0, p0)
                                      if lo < p1:
                                          segs.append(("sc", lo, p1))
                                  # start only on the bank's first write; stop
                                  # on each byte-range's final write
                                  for si, (kind, lo, hi) in enumerate(segs):
                                      st = si == 0
                                      if kind == "neg":
                                          nc.tensor.matmul(
                                              sT[:, lo:hi], ones_b[:], negrow_b[:],
                                              start=st, stop=True,
                                              skip_group_check=True,
                                          )
                                      elif kind == "diag":
                                          nc.tensor.matmul(
                                              sT[:, lo:hi],
                                              head_ap("k", h, 128 * c, 128 * c + 128),
                                              head_ap("q", h, QS * g + lo, QS * g + hi),
                                              start=st, stop=False,
                                              skip_group_check=True,
                                          )
                                          nc.tensor.matmul(
                                              sT[:, lo:hi],
                                              ident_sb[:], maskneg_sb[:],
                                              start=False, stop=True,
                                              skip_group_check=True,
                                          )
                                      else:
                                          nc.tensor.matmul(
                                              sT[:, lo:hi],
                                              head_ap("k", h, 128 * c, 128 * c + 128),
                                              head_ap("q", h, QS * g + lo, QS * g + hi),
                                              start=st, stop=True,
                                              skip_group_check=True,
                                          )
                              nc.scalar.activation(
                                  pt[:, par, pq0:QS], sT[:, pq0:QS], Exp,
                                  scale=float(SCALE),
                              )
                              if g == 0 and i == 0 and par == 0:
                                  # bf16 patch of P for q<128
                                  pt0b = wrk.tile([128, 128], bf16, tag="pt0b")
                                  nc.scalar.activation(
                                      pt0b[:], sT[:, 0:128], Exp, scale=float(SCALE)
                                  )
                                  if DEBUG and h == 0:
                                      nc.sync.dma_start(dbg["dpt0b"][:], pt0b[:])
                              if par == 0:
                                  for f in pending_b:
                                      f()
                                  pending_b = pending_b_new
                                  pending_b_new = []
                                  if prev is not None:
                                      emit_pv(*prev, ctx_=ctx, ctn_=ctn)
                              for f in sched.pop(slot, ()):
                                  f()
                              slot += 1
                          prev = (pt, pt0b, i)
                      emit_pv(*prev, ctx_=ctx, ctn_=ctn)
                      for f in pending_b + pending_b_new:
                          f()
                  if DEBUG and g == 0:
                      nc.sync.dma_start(dbg["dctn0"][:], ctn[0][:])
                      nc.sync.dma_start(dbg["dctn1"][:], ctn[1][:])
                      nc.sync.dma_start(dbg["dv8_0"][:], v8_sb[0][:])
                      nc.sync.dma_start(dbg["dv0b"][:], v0b_sb[:])
                      nc.sync.dma_start(dbg["dq0"][:], qk_sb["q"][0][:])
                      nc.sync.dma_start(dbg["dk0"][:], qk_sb["k"][0][:])
                  # out projection: g0's units are slotted into g1's
                  # attention as fillers; g1's run inline at the end.
                  if g == 0:
                      units = [
                          (jc, piece) for jc in range(6) for piece in range(2)
                      ]
                      for s, (jc, piece) in zip(OUTPROJ_SLOTS, units):
                          sched.setdefault(s, []).append(
                              lambda c_=ctn, jc_=jc, p_=piece: outproj_unit(
                                  c_, 0, jc_, p_
                              )
                          )
                  else:
                      # piece 0 first: its norms are already done, so PE
                      # stays busy while piece 1's norm chain drains on DVE
                      for piece in range(2):
                          for jc in range(6):
                              outproj_unit(ctn, g, jc, piece, alt=True)

    nc.compile()
    _cache[key] = nc
    return nc


def kernel(x, Wq, bq, Wk, bk, Wv, bv, Wo, bo):
    out, _ = run(x, Wq, bq, Wk, bk, Wv, bv, Wo, bo)
    return out


def build_in_maps(x, Wq, bq, Wk, bk, Wv, bv, Wo, bo=None):
    bf = ml_dtypes.bfloat16
    f8 = ml_dtypes.float8_e4m3
    x = np.asarray(x, np.float32)
    Wq, bq = np.asarray(Wq, np.float32), np.asarray(bq, np.float32)
    Wk, bk = np.asarray(Wk, np.float32), np.asarray(bk, np.float32)
    Wv, bv = np.asarray(Wv, np.float32), np.asarray(bv, np.float32)
    Wo = np.asarray(Wo, np.float32)

    # additive causal mask for a diagonal 128-block: 0 where q >= k, -1e9 else
    maskneg = np.where(
        np.tri(128, 128, 0, dtype=bool).T, np.float32(0.0), np.float32(NEG)
    ).astype(bf)
    # note: tri().T gives [k, q] upper-tri (q >= k -> valid -> 0)
    ident = np.eye(128, dtype=np.float32).astype(bf)

    in_maps = []
    for c in range(NCORES):
        b, rs = c // 4, (c % 4) * NH * HD
        re = rs + NH * HD
        woP = np.zeros((128, 2, D), np.float32)
        woP[:, 0, :] = Wo[:, rs : rs + 128].T
        woP[0:64, 1, :] = Wo[:, rs + 128 : rs + 192].T
        wv8 = np.zeros((D, 384), np.float32)
        bv_row = np.zeros((1, 384), np.float32)
        for h in range(NH):
            wv8[:, 128 * h + 64 : 128 * h + 128] = Wv[rs + 64 * h : rs + 64 * h + 64].T
            bv_row[0, 128 * h + 64 : 128 * h + 128] = bv[rs + 64 * h : rs + 64 * h + 64]
            bv_row[0, 128 * h] = 1.0
        xT = np.ascontiguousarray(x[b].T)
        in_maps.append(
            {
                "xT": xT.astype(bf),
                "x8T": xT.astype(f8),
                "wqT": np.ascontiguousarray(Wq[rs:re].T).astype(bf),
                "wkT": np.ascontiguousarray(Wk[rs:re].T).astype(bf),
                "wv8T": wv8.astype(f8),
                "wvT": wv8.astype(bf),
                "woT": woP.astype(bf),
                "bq01": bq[rs : rs + 128].reshape(128, 1).copy(),
                "bq2": bq[rs + 128 : re].reshape(64, 1).copy(),
                "bk01": bk[rs : rs + 128].reshape(128, 1).copy(),
                "bk2": bk[rs + 128 : re].reshape(64, 1).copy(),
                "bv": bv_row.astype(bf),
                "maskneg": maskneg,
                "ident": ident,
            }
        )
    return in_maps


def run(x, Wq, bq, Wk, bk, Wv, bv, Wo, bo, trace=False):
    from concourse.bass_utils import run_bass_kernel_spmd

    nc = _build()
    bo = np.asarray(bo, np.float32)
    in_maps = build_in_maps(x, Wq, bq, Wk, bk, Wv, bv, Wo)
    res = run_bass_kernel_spmd(nc, in_maps, list(range(NCORES)), trace=trace)
    out = np.zeros((B, S, D), np.float32)
    for b in range(B):
        acc = np.zeros((D, S), np.float32)
        for c in range(4 * b, 4 * b + 4):
            acc += res.results[c]["outT"].astype(np.float32)
        out[b] = acc.T + bo
    return out, res


# revision 61
# speedup vs baseline: 1.4271x; 1.3313x over previous
"""Causal multi-head attention (B=2, S=2048, D=768, H=12) on 8 TRN2 NeuronCores.

Sharding: core c handles batch c//4, heads 3*(c%4) .. 3*(c%4)+3.

Per core (bf16 operands, fp8e4m3 DoubleRow PV/V-proj, fp32 PSUM):
  - q/k projections bf16, chunk-major across 6 parallel PSUM groups at
    startup so PE rides the serial input-DMA stream.
  - v projection fp8 DoubleRow (256-deep contraction, 0.5 cyc/row); per-head
    128-col groups laid out [ones | 63 pad | v(64)] so the denominator lands
    on ctx partition 0 (reciprocal_approx_fast silently misreads PSUM at
    nonzero partition offsets) and v sits 64-aligned (>32-partition reads
    must be 0/64-aligned). Blocks 0-1 run bf16 (block 0 feeds the q<128
    patch; block 1 hides the x8 DMA arrival).
  - scores TRANSPOSED sT[k,q] = K.Q^T bf16; causal mask ACCUMULATED into the
    diagonal 128-block via identity-lhsT matmul; odd chunks of a k-pair get
    a rank-1 -1e9 fill so the pair-exp writes exact zeros. Exactly one
    start=True per PSUM bank (start zeroes the whole 2KB region) and
    stop=True on each byte-range's final write.
  - exp on ACT straight from PSUM -> P^T fp8 in [128,2,QS] pair tiles; PV is
    fp8 DoubleRow over k-pairs, emitted one pair late so it never waits on
    exp. q<128 is patched with a bf16 exp + bf16 PV (fp8 softmax-weight
    error spikes at tiny Neff). Remaining projections/outproj work is
    slot-scheduled into the attention loop as PE fillers.
  - normalization: DVE reciprocal_approx_fast reads the denominator row from
    PSUM; bf16 copy on GpSimd; ones-matmul broadcast; DVE multiply -> bf16
    ctn. The PE-visible stage is deferred two pairs to hide the chain.
  - out-projection bf16; g0's units interleave into g1 as fillers; outputs
    stored bf16 on alternating HWDGE queues, summed across cores on host.
"""

import numpy as np
import ml_dtypes

B, S, D, H, HD = 2, 2048, 768, 12, 64
NH = 3                      # heads per core
NCORES = 8
SCALE = 1.0 / np.sqrt(HD)
QS = 1024                   # q superblock width
NG = S // QS                # 2 q superblocks
NKC = S // 128              # 16 k chunks
NXC = D // 128              # 6 contraction chunks of 128 over D
NEG = -1.0e9                # causal mask fill (exp(NEG*SCALE) == 0)

_cache = {}
DEBUG = False


def _enable_ldw_opt():
    """No-op: walrus's ldw-opt rejects the explicit InstLdweights that
    bf16/fp8 matmuls emit (it only composes with f32r's implicit loads)."""


def _build(reps=1):
    _enable_ldw_opt()
    key = ("nc", reps)
    if key in _cache:
        return _cache[key]
    import concourse.bacc as bacc
    import concourse.mybir as mybir
    import concourse.tile as tile

    f32 = mybir.dt.float32
    bf16 = mybir.dt.bfloat16
    fp8 = mybir.dt.float8e4
    DR = mybir.MatmulPerfMode.DoubleRow
    Exp = mybir.ActivationFunctionType.Exp
    add_op = mybir.AluOpType.add

    nc = bacc.Bacc(None, target_bir_lowering=False, debug=False, num_devices=NCORES)

    xT_d = nc.dram_tensor("xT", [D, S], bf16, kind="ExternalInput")
    x8T_d = nc.dram_tensor("x8T", [D, S], fp8, kind="ExternalInput")
    wqT_d = nc.dram_tensor("wqT", [D, NH * HD], bf16, kind="ExternalInput")
    wkT_d = nc.dram_tensor("wkT", [D, NH * HD], bf16, kind="ExternalInput")
    wv8T_d = nc.dram_tensor("wv8T", [D, 384], fp8, kind="ExternalInput")
    wvT_d = nc.dram_tensor("wvT", [D, 384], bf16, kind="ExternalInput")
    woT_d = nc.dram_tensor("woT", [128, 2, D], bf16, kind="ExternalInput")
    bq01_d = nc.dram_tensor("bq01", [128, 1], f32, kind="ExternalInput")
    bq2_d = nc.dram_tensor("bq2", [64, 1], f32, kind="ExternalInput")
    bk01_d = nc.dram_tensor("bk01", [128, 1], f32, kind="ExternalInput")
    bk2_d = nc.dram_tensor("bk2", [64, 1], f32, kind="ExternalInput")
    bv_d = nc.dram_tensor("bv", [1, 384], bf16, kind="ExternalInput")
    maskneg_d = nc.dram_tensor("maskneg", [128, 128], bf16, kind="ExternalInput")
    ident_d = nc.dram_tensor("ident", [128, 128], bf16, kind="ExternalInput")
    outT_d = nc.dram_tensor("outT", [D, S], bf16, kind="ExternalOutput")
    dbg = {}
    if DEBUG:
        import concourse.mybir as _mb
        dbg["dpt0"] = nc.dram_tensor("dpt0", [128, 2, QS], fp8, kind="ExternalOutput")
        dbg["dv8_0"] = nc.dram_tensor("dv8_0", [128, 2, NH, 128], fp8, kind="ExternalOutput")
        dbg["dv0b"] = nc.dram_tensor("dv0b", [128, NH, 128], bf16, kind="ExternalOutput")
        dbg["drec0"] = nc.dram_tensor("drec0", [1, 512], f32, kind="ExternalOutput")
        dbg["drec1"] = nc.dram_tensor("drec1", [1, 512], f32, kind="ExternalOutput")
        dbg["dq0"] = nc.dram_tensor("dq0", [128, 2, 512], bf16, kind="ExternalOutput")
        dbg["dk0"] = nc.dram_tensor("dk0", [128, 2, 512], bf16, kind="ExternalOutput")
        dbg["dctn0"] = nc.dram_tensor("dctn0", [128, 2, 512], bf16, kind="ExternalOutput")
        dbg["dcts0"] = nc.dram_tensor("dcts0", [64, 512], bf16, kind="ExternalOutput")
        dbg["dden0"] = nc.dram_tensor("dden0", [1, 512], f32, kind="ExternalOutput")
        dbg["dpt0b"] = nc.dram_tensor("dpt0b", [128, 128], bf16, kind="ExternalOutput")
        dbg["dctn1"] = nc.dram_tensor("dctn1", [128, 2, 512], bf16, kind="ExternalOutput")

    with tile.TileContext(nc) as tc:
        with (
            tc.tile_pool(name="const", bufs=1) as cst,
            tc.tile_pool(name="work", bufs=3) as wrk,
            tc.tile_pool(name="norm", bufs=2) as nrm,
            tc.tile_pool(name="ps_sT", bufs=2, space="PSUM") as ps_sT,
            tc.tile_pool(name="ps_ctx", bufs=1, space="PSUM") as ps_ctx,
            tc.tile_pool(name="ps_mm", bufs=2, space="PSUM") as ps_mm,
        ):
         for _rep in range(reps):
              # ---- constant / persistent SBUF ----
              # Small consts on the gpsimd SWDGE queue (its ~1.1us/DMA serial
              # generation cost is fine for these); weights + x interleaved
              # across the two HWDGE queues (SP, ACT) ordered by first use.
              maskneg_sb = cst.tile([128, 128], bf16)
              nc.gpsimd.dma_start(maskneg_sb[:], maskneg_d[:])
              ident_sb = cst.tile([128, 128], bf16)
              nc.gpsimd.dma_start(ident_sb[:], ident_d[:])
              bq01 = cst.tile([128, 1], f32)
              nc.gpsimd.dma_start(bq01[:], bq01_d[:])
              bq2 = cst.tile([64, 1], f32)
              nc.gpsimd.dma_start(bq2[:], bq2_d[:])
              bk01 = cst.tile([128, 1], f32)
              nc.gpsimd.dma_start(bk01[:], bk01_d[:])
              bk2 = cst.tile([64, 1], f32)
              nc.gpsimd.dma_start(bk2[:], bk2_d[:])
              bv_sb = cst.tile([1, 384], bf16)
              nc.gpsimd.dma_start(bv_sb[:], bv_d[:])

              # DMA engines serialize at HBM bandwidth, so submission order ==
              # arrival order: weights first, then x chunk-by-chunk (the
              # chunk-major initial projections ride this stream), then the
              # late-use tensors (wv bf16, wo, x8, wv8).
              xT_r = xT_d[:].rearrange("(c p) s -> p c s", p=128)
              x8_r = x8T_d[:].rearrange("(c p) s -> p c s", p=128)
              wq_sb = cst.tile([128, NXC, NH * HD], bf16)
              nc.sync.dma_start(wq_sb[:], wqT_d[:].rearrange("(c p) m -> p c m", p=128))
              wk_sb = cst.tile([128, NXC, NH * HD], bf16)
              nc.scalar.dma_start(wk_sb[:], wkT_d[:].rearrange("(c p) m -> p c m", p=128))
              x_sb = []
              for c in range(NXC):
                  xc = cst.tile([128, S], bf16, tag=f"x{c}")
                  eng = nc.sync if c % 2 == 0 else nc.scalar
                  eng.dma_start(xc[:], xT_r[:, c, :])
                  x_sb.append(xc)
              wv_sb = cst.tile([128, NXC, 384], bf16)
              nc.scalar.dma_start(wv_sb[:], wvT_d[:].rearrange("(c p) m -> p c m", p=128))
              x8_sb = cst.tile([128, NXC, S], fp8)
              nc.sync.dma_start(x8_sb[:, 0:3, :], x8_r[:, 0:3, :])
              nc.scalar.dma_start(x8_sb[:, 3:6, :], x8_r[:, 3:6, :])
              wv8_sb = cst.tile([128, NXC, 384], fp8)
              nc.sync.dma_start(wv8_sb[:], wv8T_d[:].rearrange("(c p) m -> p c m", p=128))
              wo_sb = cst.tile([128, 2, D], bf16)
              nc.scalar.dma_start(wo_sb[:], woT_d[:])

              ones_f = cst.tile([1, 128], f32)
              nc.vector.memset(ones_f[:], 1.0)
              ones_b = cst.tile([1, 128], bf16)
              nc.vector.tensor_copy(ones_b[:], ones_f[:])
              ones_r = cst.tile([1, 128], mybir.dt.float32r)
              nc.vector.tensor_copy(ones_r[:], ones_f[:])
              negrow_f = cst.tile([1, 128], f32)
              nc.vector.memset(negrow_f[:], NEG)
              negrow_b = cst.tile([1, 128], bf16)
              nc.vector.tensor_copy(negrow_b[:], negrow_f[:])

              # persistent activations:
              # q/k: one tile per 512-wide s-super; slot 0 holds heads 0/1
              # stacked on partitions, slot 1 head 2.
              qk_sb = {
                  t: [
                      cst.tile(
                          [128, 2, 512], bf16, tag=f"{t}sp{sp}", name=f"{t}sp{sp}"
                      )
                      for sp in range(4)
                  ]
                  for t in ("q", "k")
              }
              # v_aug fp8 per k-pair: [128, 2, NH, 65]; col 64 of each head
              # group is the ones column (from the bias matmul's bv layout).
              v8_sb = [
                  cst.tile([128, 2, NH, 128], fp8, tag=f"vp{i}", name=f"vp{i}")
                  for i in range(NKC // 2)
              ]
              # bf16 copy of v block 0 for the q<128 patch
              v0b_sb = cst.tile([128, NH, 128], bf16)

              def head_ap(t, h, lo, hi):
                  """AP for head h, global columns [lo, hi) (within one super)."""
                  sp, o = lo // 512, lo % 512
                  tile_ = qk_sb[t][sp]
                  if h < 2:
                      return tile_[64 * h : 64 * h + 64, 0, o : o + hi - lo]
                  return tile_[0:64, 1, o : o + hi - lo]

              def qk_proj(t, sp, mi):
                  scols = slice(512 * sp, 512 * sp + 512)
                  m0, msz, slot = ((0, 128, 0), (128, 64, 1))[mi]
                  b01, b2 = (bq01, bq2) if t == "q" else (bk01, bk2)
                  p = ps_mm.tile([128, 512], f32, tag="mm")
                  for c in range(NXC):
                      nc.tensor.matmul(
                          p[:msz, :],
                          (wq_sb if t == "q" else wk_sb)[:, c, m0 : m0 + msz],
                          x_sb[c][:, scols],
                          start=(c == 0),
                          stop=(c == NXC - 1),
                      )
                  nc.vector.tensor_scalar(
                      out=qk_sb[t][sp][:msz, slot, :],
                      in0=p[:msz, :],
                      scalar1=(b01 if mi == 0 else b2)[:msz],
                      scalar2=None,
                      op0=add_op,
                  )

              def v_proj(blk):
                  # fp8 DoubleRow over 3 contraction pairs; the bf16 rank-1
                  # bias matmul adds bv + the ones column of each head group.
                  # Block 0 runs bf16: it feeds the q<128 patch, whose whole
                  # point is bf16-quality v for the tiny-Neff positions.
                  p = ps_mm.tile([128, 384], f32, tag="mm")
                  if blk < 2:
                      for c in range(NXC):
                          nc.tensor.matmul(
                              p[:],
                              x_sb[c][:, 128 * blk : 128 * blk + 128],
                              wv_sb[:, c, :],
                              start=(c == 0),
                              stop=False,
                          )
                  else:
                      for i in range(3):
                          nc.tensor.matmul(
                              p[:],
                              x8_sb[:, 2 * i : 2 * i + 2, 128 * blk : 128 * blk + 128],
                              wv8_sb[:, 2 * i : 2 * i + 2, :],
                              start=(i == 0),
                              stop=False,
                              perf_mode=DR,
                          )
                  nc.tensor.matmul(p[:], ones_b[:], bv_sb[:], start=False, stop=True)
                  nc.vector.tensor_copy(
                      v8_sb[blk // 2][:, blk % 2, :, :], p[:, 0 : NH * 128]
                  )
                  if blk == 0:
                      nc.vector.tensor_copy(v0b_sb[:], p[:, 0 : NH * 128])

              # ---- initial projections: just enough for g=0 h=0 ----
              # Chunk-major across 6 parallel PSUM accumulation groups
              # (borrowing slots from every pool) so PE consumes each x chunk
              # the moment it lands instead of stalling group-by-group.
              igroups = [
                  ("q", 0, 0), ("q", 1, 0), ("k", 0, 0),
                  ("q", 1, 1), ("q", 0, 1), ("k", 0, 1),
              ]
              ipsum = [
                  ps_mm.tile([128, 512], f32, tag="mm", name="ip0"),
                  ps_mm.tile([128, 512], f32, tag="mm", name="ip1"),
                  ps_sT.tile([128, QS], f32, tag="sT", name="ip2"),
                  ps_sT.tile([128, QS], f32, tag="sT", name="ip3"),
                  ps_ctx.tile([128, 512], f32, tag="ctx0", name="ip4"),
                  ps_ctx.tile([128, 512], f32, tag="ctx1", name="ip5"),
              ]
              for c in range(NXC):
                  for gi, (t, sp, mi) in enumerate(igroups):
                      m0, msz, _ = ((0, 128, 0), (128, 64, 1))[mi]
                      nc.tensor.matmul(
                          ipsum[gi][:msz, 0:512],
                          (wq_sb if t == "q" else wk_sb)[:, c, m0 : m0 + msz],
                          x_sb[c][:, 512 * sp : 512 * sp + 512],
                          start=(c == 0),
                          stop=(c == NXC - 1),
                      )
              for gi, (t, sp, mi) in enumerate(igroups):
                  m0, msz, slot = ((0, 128, 0), (128, 64, 1))[mi]
                  b01, b2 = (bq01, bq2) if t == "q" else (bk01, bk2)
                  nc.vector.tensor_scalar(
                      out=qk_sb[t][sp][:msz, slot, :],
                      in0=ipsum[gi][:msz, 0:512],
                      scalar1=(b01 if mi == 0 else b2)[:msz],
                      scalar2=None,
                      op0=add_op,
                  )
              qk_proj("k", 1, 0)

              # remaining projection + g0-outproj work, interleaved into the
              # attention loop at fixed chunk slots (slot = global chunk
              # counter 0..71) chosen to meet each consumer's deadline while
              # spreading PE work into the ACT-limited stretches.
              def qk(t, sp, mi):
                  return lambda: qk_proj(t, sp, mi)

              def vp(b):
                  return lambda: v_proj(b)

              sched = {
                  0: [vp(0)], 1: [vp(1)], 2: [vp(2)], 3: [vp(3)],
                  4: [vp(4)], 5: [vp(5)], 6: [vp(6)], 7: [vp(7)],
                  10: [qk("q", 2, 0)], 12: [qk("q", 2, 1)],
                  14: [qk("q", 3, 0)], 16: [qk("q", 3, 1)],
                  18: [qk("k", 1, 1)],
                  26: [qk("k", 2, 0)], 28: [vp(8)], 30: [vp(9)],
                  33: [qk("k", 3, 0)], 34: [vp(10)], 35: [vp(11)],
                  36: [vp(12)], 37: [vp(13)], 38: [vp(14)], 39: [vp(15)],
                  56: [qk("k", 2, 1)], 60: [qk("k", 3, 1)],
              }
              # g0 outproj units get slotted into g1 once ctn_g0 exists
              OUTPROJ_SLOTS = (40, 43, 46, 49, 52, 55, 57, 59, 61, 63, 65, 67)

              def outproj_unit(ctn, g, jc, piece, alt=False):
                  # alternate the PSUM ring (tail only — sT ring is live
                  # during attention) and the PSUM->SBUF copy engine so the
                  # drain isn't serialized on one resource
                  if alt and (jc + piece) % 2:
                      po = ps_sT.tile([128, 512], f32, tag="sT", name=f"po_{jc}_{piece}")
                  else:
                      po = ps_mm.tile([128, 512], f32, tag="mm")
                  nc.tensor.matmul(
                      po[:],
                      wo_sb[:, 0, 128 * jc : 128 * jc + 128],
                      ctn[piece][:, 0, :],
                      start=True,
                      stop=False,
                  )
                  nc.tensor.matmul(
                      po[:],
                      wo_sb[0:64, 1, 128 * jc : 128 * jc + 128],
                      ctn[piece][0:64, 1, :],
                      start=False,
                      stop=True,
                  )
                  ot = wrk.tile([128, 512], bf16, tag="ot", bufs=8)
                  if alt and (jc + piece) % 2:
                      # tail only: ACT is idle once the exps are done
                      nc.scalar.copy(ot[:], po[:])
                  else:
                      nc.vector.tensor_copy(ot[:], po[:])
                  dma_eng = nc.scalar if (jc + piece) % 2 else nc.sync
                  dma_eng.dma_start(
                      outT_d[
                          128 * jc : 128 * jc + 128,
                          QS * g + 512 * piece : QS * g + 512 * piece + 512,
                      ],
                      ot[:],
                  )

              # ---- attention + out-projection per q superblock ----
              slot = 0
              for g in range(NG):
                  # normalized ctxT per 512-piece, bf16
                  # packed: [0:64,0]=h0, [64:128,0]=h1, [0:64,1]=h2
                  ctn = [
                      nrm.tile([128, 2, 512], bf16, tag=f"ctn{p}", name=f"ctn{p}_{g}")
                      for p in range(2)
                  ]
                  for h in range(NH):
                      ctx = [
                          ps_ctx.tile(
                              [128, 512], f32, tag=f"ctx{p}", name=f"ctx{p}_{g}_{h}"
                          )
                          for p in range(2)
                      ]
                      kpairs = 4 * (g + 1)

                      def emit_pv(pt_, pb_, i_, g_=g, h_=h, ctx_=None, ctn_=None):
                          # PV: fp8 DoubleRow over pair i_ (emitted one pair
                          # late so it never waits on the pair's exp).
                          # One start-chain per ctx bank: the bf16 q<128 patch
                          # is the bank's opening write in (g0, pair0, piece0).
                          if g_ == 0 and i_ == 0:
                              nc.tensor.matmul(
                                  ctx_[0][:, 0:128], v0b_sb[:, h_, :], pb_[:],
                                  start=True, stop=False,
                                  skip_group_check=True,
                              )
                          if DEBUG and g_ == 0 and h_ == 0 and i_ == 0:
                              nc.sync.dma_start(dbg["dpt0"][:], pt_[:])
                          pq0_ = max(0, 128 * (2 * i_ - 8 * g_))
                          for piece in range(2):
                              lp = 2 * piece + 1 + 4 * g_
                              lo = max(pq0_, 512 * piece)
                              patch0 = g_ == 0 and piece == 0
                              if patch0 and i_ == 0:
                                  lo = 128  # q<128 handled by the bf16 patch
                              hi = 512 * piece + 512
                              if lo >= hi:
                                  continue
                              nc.tensor.matmul(
                                  ctx_[piece][:, lo - 512 * piece : hi - 512 * piece],
                                  v8_sb[i_][:, :, h_, :],
                                  pt_[:, :, lo:hi],
                                  start=(i_ == 0 and not patch0),
                                  stop=(i_ == lp),
                                  perf_mode=DR,
                                  skip_group_check=True,
                              )
                          # normalization stage A (DVE/Pool reciprocal
                          # chain) per piece as soon as it stops; stage B
                          # (PE broadcast + multiply) is deferred one pair so
                          # PE never waits on the chain
                          for piece in range(2):
                              if i_ != 2 * piece + 1 + 4 * g_:
                                  continue
                              rec = nrm.tile([1, 512], f32, tag="rec")
                              nc.vector.reciprocal_approx_fast(
                                  out=rec[:], in_=ctx_[piece][0:1, :]
                              )
                              if DEBUG and g_ == 0 and h_ == 0:
                                  nc.sync.dma_start(dbg[f"drec{piece}"][:], rec[:])
                              recb = nrm.tile([1, 512], bf16, tag="recb")
                              nc.vector.tensor_copy(recb[:], rec[:])

                              def norm_b(piece=piece, recb=recb, g_=g_, h_=h_,
                                         ctx_=ctx_, ctn_=ctn_):
                                  bc = ps_mm.tile([64, 512], f32, tag="mm")
                                  nc.tensor.matmul(
                                      bc[:], ones_b[:, 0:64], recb[:],
                                      start=True, stop=True,
                                  )
                                  cts = nrm.tile([64, 512], bf16, tag="cts")
                                  nc.vector.tensor_copy(
                                      cts[:], ctx_[piece][64:128, :]
                                  )
                                  dst = (
                                      ctn_[piece][64 * h_ : 64 * h_ + 64, 0, :]
                                      if h_ < 2
                                      else ctn_[piece][0:64, 1, :]
                                  )
                                  nc.vector.tensor_mul(dst, cts[:], bc[:])

                              pending_b_new.append(norm_b)

                      prev = None  # (pt, pt0b, pair) awaiting PV emission
                      pt0b = None
                      pending_b = []      # norm stage-B ready to emit
                      pending_b_new = []  # norm stage-B queued one extra pair
                      for i in range(kpairs):
                          pq0 = max(0, 128 * (2 * i - 8 * g))  # pair's first valid col
                          pt = wrk.tile([128, 2, QS], fp8, tag="pt")
                          for par in range(2):
                              c = 2 * i + par
                              j = c - 8 * g
                              q0 = max(0, 128 * j)
                              sT = ps_sT.tile([128, QS], f32, tag="sT")
                              # PSUM accumulation is tracked per 2KB bank:
                              # exactly ONE start=True (first write) and one
                              # stop=True (last write) per 512-col piece, and
                              # every byte exp reads must be written after the
                              # start (pending-zero bytes read back garbage).
                              for piece in range(2):
                                  p0, p1 = 512 * piece, 512 * piece + 512
                                  segs = []  # (kind, lo, hi)
                                  if par == 1 and q0 > pq0:
                                      # odd chunk's leading masked cols: rank-1
                                      # -1e9 fill so the pair-exp writes zeros
                                      nl, nh = max(pq0, p0), min(q0, p1)
                                      if nl < nh:
                                          segs.append(("neg", nl, nh))
                                  if j >= 0 and p0 <= q0 < p1:
                                      segs.append(("diag", q0, q0 + 128))
                                      if q0 + 128 < p1:
                                          segs.append(("sc", q0 + 128, p1))
                                  else:
                                      lo = max(q0, p0)
                                      if lo < p1:
                                          segs.append(("sc", lo, p1))
                                  # start only on the bank's first write; stop
                                  # on each byte-range's final write
                                  for si, (kind, lo, hi) in enumerate(segs):
                                      st = si == 0
                                      if kind == "neg":
                                          nc.tensor.matmul(
                                              sT[:, lo:hi], ones_b[:], negrow_b[:],
                                              start=st, stop=True,
                                              skip_group_check=True,
                                          )
                                      elif kind == "diag":
                                          nc.tensor.matmul(
                                              sT[:, lo:hi],
                                              head_ap("k", h, 128 * c, 128 * c + 128),
                                              head_ap("q", h, QS * g + lo, QS * g + hi),
                                              start=st, stop=False,
                                              skip_group_check=True,
                                          )
                                          nc.tensor.matmul(
                                              sT[:, lo:hi],
                                              ident_sb[:], maskneg_sb[:],
                                              start=False, stop=True,
                                              skip_group_check=True,
                                          )
                                      else:
                                          nc.tensor.matmul(
                                              sT[:, lo:hi],
                                              head_ap("k", h, 128 * c, 128 * c + 128),
                                              head_ap("q", h, QS * g + lo, QS * g + hi),
                                              start=st, stop=True,
                                              skip_group_check=True,
                                          )
                              nc.scalar.activation(
                                  pt[:, par, pq0:QS], sT[:, pq0:QS], Exp,
                                  scale=float(SCALE),
                              )
                              if g == 0 and i == 0 and par == 0:
                                  # bf16 patch of P for q<128
                                  pt0b = wrk.tile([128, 128], bf16, tag="pt0b")
                                  nc.scalar.activation(
                                      pt0b[:], sT[:, 0:128], Exp, scale=float(SCALE)
                                  )
                                  if DEBUG and h == 0:
                                      nc.sync.dma_start(dbg["dpt0b"][:], pt0b[:])
                              if par == 0:
                                  for f in pending_b:
                                      f()
                                  pending_b = pending_b_new
                                  pending_b_new = []
                                  if prev is not None:
                                      emit_pv(*prev, ctx_=ctx, ctn_=ctn)
                              for f in sched.pop(slot, ()):
                                  f()
                              slot += 1
                          prev = (pt, pt0b, i)
                      emit_pv(*prev, ctx_=ctx, ctn_=ctn)
                      for f in pending_b + pending_b_new:
                          f()
                  if DEBUG and g == 0:
                      nc.sync.dma_start(dbg["dctn0"][:], ctn[0][:])
                      nc.sync.dma_start(dbg["dctn1"][:], ctn[1][:])
                      nc.sync.dma_start(dbg["dv8_0"][:], v8_sb[0][:])
                      nc.sync.dma_start(dbg["dv0b"][:], v0b_sb[:])
                      nc.sync.dma_start(dbg["dq0"][:], qk_sb["q"][0][:])
                      nc.sync.dma_start(dbg["dk0"][:], qk_sb["k"][0][:])
                  # out projection: g0's units are slotted into g1's
                  # attention as fillers; g1's run inline at the end.
                  if g == 0:
                      units = [
                          (jc, piece) for jc in range(6) for piece in range(2)
                      ]
                      for s, (jc, piece) in zip(OUTPROJ_SLOTS, units):
                          sched.setdefault(s, []).append(
                              lambda c_=ctn, jc_=jc, p_=piece: outproj_unit(
                                  c_, 0, jc_, p_
                              )
                          )
                  else:
                      # piece 0 first: its norms are already done, so PE
                      # stays busy while piece 1's norm chain drains on DVE
                      for piece in range(2):
                          for jc in range(6):
                              outproj_unit(ctn, g, jc, piece, alt=True)

    nc.compile()
    _cache[key] = nc
    return nc


def kernel(x, Wq, bq, Wk, bk, Wv, bv, Wo, bo):
    out, _ = run(x, Wq, bq, Wk, bk, Wv, bv, Wo, bo)
    return out


def build_in_maps(x, Wq, bq, Wk, bk, Wv, bv, Wo, bo=None):
    bf = ml_dtypes.bfloat16
    f8 = ml_dtypes.float8_e4m3
    x = np.asarray(x, np.float32)
    Wq, bq = np.asarray(Wq, np.float32), np.asarray(bq, np.float32)
    Wk, bk = np.asarray(Wk, np.float32), np.asarray(bk, np.float32)
    Wv, bv = np.asarray(Wv, np.float32), np.asarray(bv, np.float32)
    Wo = np.asarray(Wo, np.float32)

    # additive causal mask for a diagonal 128-block: 0 where q >= k, -1e9 else
    maskneg = np.where(
        np.tri(128, 128, 0, dtype=bool).T, np.float32(0.0), np.float32(NEG)
    ).astype(bf)
    # note: tri().T gives [k, q] upper-tri (q >= k -> valid -> 0)
    ident = np.eye(128, dtype=np.float32).astype(bf)

    in_maps = []
    for c in range(NCORES):
        b, rs = c // 4, (c % 4) * NH * HD
        re = rs + NH * HD
        woP = np.zeros((128, 2, D), np.float32)
        woP[:, 0, :] = Wo[:, rs : rs + 128].T
        woP[0:64, 1, :] = Wo[:, rs + 128 : rs + 192].T
        wv8 = np.zeros((D, 384), np.float32)
        bv_row = np.zeros((1, 384), np.float32)
        for h in range(NH):
            wv8[:, 128 * h + 64 : 128 * h + 128] = Wv[rs + 64 * h : rs + 64 * h + 64].T
            bv_row[0, 128 * h + 64 : 128 * h + 128] = bv[rs + 64 * h : rs + 64 * h + 64]
            bv_row[0, 128 * h] = 1.0
        xT = np.ascontiguousarray(x[b].T)
        in_maps.append(
            {
                "xT": xT.astype(bf),
                "x8T": xT.astype(f8),
                "wqT": np.ascontiguousarray(Wq[rs:re].T).astype(bf),
                "wkT": np.ascontiguousarray(Wk[rs:re].T).astype(bf),
                "wv8T": wv8.astype(f8),
                "wvT": wv8.astype(bf),
                "woT": woP.astype(bf),
                "bq01": bq[rs : rs + 128].reshape(128, 1).copy(),
                "bq2": bq[rs + 128 : re].reshape(64, 1).copy(),
                "bk01": bk[rs : rs + 128].reshape(128, 1).copy(),
                "bk2": bk[rs + 128 : re].reshape(64, 1).copy(),
                "bv": bv_row.astype(bf),
                "maskneg": maskneg,
                "ident": ident,
            }
        )
    return in_maps


def run(x, Wq, bq, Wk, bk, Wv, bv, Wo, bo, trace=False):
    from concourse.bass_utils import run_bass_kernel_spmd

    nc = _build()
    bo = np.asarray(bo, np.float32)
    in_maps = build_in_maps(x, Wq, bq, Wk, bk, Wv, bv, Wo)
    res = run_bass_kernel_spmd(nc, in_maps, list(range(NCORES)), trace=trace)
    out = np.zeros((B, S, D), np.float32)
    for b in range(B):
        acc = np.zeros((D, S), np.float32)
        for c in range(4 * b, 4 * b + 4):
            acc += res.results[c]["outT"].astype(np.float32)
        out[b] = acc.T + bo
    return out, res
